# revision 1
# baseline (speedup 1.0000x reference)
"""Trainium2 Bass kernel for nn_Classifier_52166672777735.

Strategy (8 NeuronCores, SPMD):
  - Pooling: data-parallel over batch B (core r pools x[r], 9.6MB DMA each),
    one feats AllGather.
  - encoder_n/encoder_v/fuse_ev collapsed host-side into two weight mats
    (pure linear algebra), output sliced per core (core r computes its own
    128-wide h0/c0 hidden chunk).  1/49 mean-pool scale folded into weights.
  - LSTM: tensor-parallel over the hidden dim.  Core r owns hidden units
    [128r, 128r+128) and computes gate columns [i_r|f_r|o_r|g_r] for all
    (t,b) rows; per-step AllGather of transposed hidden chunks.
    Ragged: position i needs T-i steps; rows sorted t-major so active rows
    are a shrinking prefix; retired rows freeze (their h is last_h).
  - Classifier: Wc/LayerNorm/relu replicated, Wa split column-wise.
Precision: weights/hidden-exchange in bf16, small matmuls in float32r
(TF32-like, 1 cycle/row vs 4 for fp32 on the PE); cell state, gate
accumulation (PSUM) and LayerNorm statistics in full fp32.
The xpre bias term is folded into the gate PSUM group via an identity
matmul so ScalarE reads gate preactivations straight from PSUM.
HW-verified rel err 3.6e-3 vs the fp32 reference (gate: 2e-2).
"""
import sys
import numpy as np

sys.path.insert(0, "/opt/trn_rl_repo")

from concourse import bass, bacc, tile, mybir  # noqa: E402
from concourse.bass_utils import run_bass_kernel_spmd  # noqa: E402

F32 = mybir.dt.float32
AF = mybir.ActivationFunctionType
ALU = mybir.AluOpType

D = 1024
NUM_A = 1887
B, T, H, W = 8, 16, 7, 7
NC = 8
CH = D // NC            # 128, per-core hidden chunk
ROWS = T * B            # 128 LSTM rows, t-major (row = t*B + b)
NA_PAD = 256            # per-core classifier cols (8*256 = 2048 >= 1887; 256 keeps fp32r matmul on the fast path)
HWST = H * W * 3        # 147
KC = D // 128           # 8 contraction chunks

_CACHE = {}

F32R = mybir.dt.float32r
BF16 = mybir.dt.bfloat16


def _mm(nc, out, lhsT, rhs, **kw):
    """matmul; f32 operands viewed as float32r (TF32-like, 4x faster than f32)."""
    if lhsT.dtype == F32:
        lhsT = lhsT.bitcast(F32R)
    if rhs.dtype == F32:
        rhs = rhs.bitcast(F32R)
    nc.tensor.matmul(out, lhsT, rhs, **kw)


def _tr(nc, out, in_, ident, **kw):
    nc.tensor.transpose(out.bitcast(F32R), in_.bitcast(F32R), ident.bitcast(F32R), **kw)



def _exw(s):
    """exchange width at exchange index s (s = 0..16)."""
    return ROWS if s == 0 else ROWS + B - B * s


def build_program(trace_names=False):
    nc = bacc.Bacc("TRN2", target_bir_lowering=False, debug=False,
                   enable_asserts=True, num_devices=NC)

    # ---------------- I/O ----------------
    xs = nc.dram_tensor("xs", [T, D, HWST], F32, kind="ExternalInput")
    ident = nc.dram_tensor("ident", [128, 128], F32R, kind="ExternalInput")
    ones = nc.dram_tensor("ones", [1, 128], F32R, kind="ExternalInput")
    wnfT = nc.dram_tensor("wnfT", [KC, 128, 2 * CH], BF16, kind="ExternalInput")
    wvfT = nc.dram_tensor("wvfT", [KC, 128, 2 * CH], BF16, kind="ExternalInput")
    bffT = nc.dram_tensor("bffT", [1, 2 * CH], F32R, kind="ExternalInput")
    wihT = nc.dram_tensor("wihT", [KC, 128, 4 * CH], BF16, kind="ExternalInput")
    bihhT = nc.dram_tensor("bihhT", [1, 4 * CH], F32R, kind="ExternalInput")
    whhT = nc.dram_tensor("whhT", [KC, 128, 4 * CH], BF16, kind="ExternalInput")
    wcT = nc.dram_tensor("wcT", [KC, 128, D], BF16, kind="ExternalInput")
    bcT = nc.dram_tensor("bcT", [1, D], F32R, kind="ExternalInput")
    waT = nc.dram_tensor("waT", [KC, 128, NA_PAD], F32R, kind="ExternalInput")
    baT = nc.dram_tensor("baT", [1, NA_PAD], F32R, kind="ExternalInput")
    gam = nc.dram_tensor("gam", [128, D], F32, kind="ExternalInput")
    bet = nc.dram_tensor("bet", [128, D], F32, kind="ExternalInput")
    out = nc.dram_tensor("out", [ROWS, NA_PAD], F32, kind="ExternalOutput")

    # internal DRAM for collectives
    fbounce = nc.dram_tensor("fbounce", [KC * 128, T * 3], BF16, kind="Internal")
    fgather = nc.dram_tensor("fgather", [NC * KC * 128, T * 3], BF16,
                             kind="Internal", addr_space="Shared")
    ccin = [nc.dram_tensor(f"ccin{s}", [128, _exw(s)], BF16, kind="Internal")
            for s in range(T + 1)]
    ccout = [nc.dram_tensor(f"ccout{s}", [NC * 128, _exw(s)], BF16,
                            kind="Internal", addr_space="Shared")
             for s in range(T + 1)]

    with tile.TileContext(nc) as tc:
        with (
            tc.tile_pool(name="w", bufs=1) as wpool,       # resident weights
            tc.tile_pool(name="xin", bufs=3) as xpool,     # streaming x tiles
            tc.tile_pool(name="st", bufs=1) as spool,      # persistent state
            tc.tile_pool(name="wk", bufs=3) as kpool,      # working tiles
            tc.tile_pool(name="ps", bufs=2, space="PSUM") as ppool,
        ):
            # ---- resident small tiles ----
            ident_sb = wpool.tile([128, 128], F32R, tag="ident")
            ones_sb = wpool.tile([1, 128], F32R, tag="ones")
            nc.sync.dma_start(ident_sb[:], ident.ap())
            nc.sync.dma_start(ones_sb[:], ones.ap())

            # ---- phase A weights ----
            wnf_sb = wpool.tile([128, KC, 2 * CH], BF16, tag="wnf")
            wvf_sb = wpool.tile([128, KC, 2 * CH], BF16, tag="wvf")
            wih_sb = wpool.tile([128, KC, 4 * CH], BF16, tag="wih")
            whh_sb = wpool.tile([128, KC, 4 * CH], BF16, tag="whh")
            bff_sb = wpool.tile([1, 2 * CH], F32R, tag="bff")
            bihh_sb = wpool.tile([1, 4 * CH], F32R, tag="bihh")
            nc.sync.dma_start(wnf_sb[:], wnfT.ap().rearrange("k p n -> p k n"))
            nc.sync.dma_start(wvf_sb[:], wvfT.ap().rearrange("k p n -> p k n"))
            nc.sync.dma_start(wih_sb[:], wihT.ap().rearrange("k p n -> p k n"))
            nc.sync.dma_start(whh_sb[:], whhT.ap().rearrange("k p n -> p k n"))
            nc.sync.dma_start(bff_sb[:], bffT.ap())
            nc.sync.dma_start(bihh_sb[:], bihhT.ap())

            # ---- pooling: x -> pooled (sum over HxW), [128, c, t, st] ----
            pooled = spool.tile([128, KC, T, 3], F32, tag="pooled")
            for c in range(KC):
                xt = xpool.tile([128, T, HWST], F32, tag="xt")
                nc.sync.dma_start(
                    xt[:], xs.ap()[:, c * 128:(c + 1) * 128, :].rearrange("t p f -> p t f"))
                nc.vector.tensor_reduce(
                    pooled[:, c], xt[:].rearrange("p t (h st) -> p t st h", st=3),
                    axis=mybir.AxisListType.X, op=ALU.add)

            # ---- feats AllGather ----
            pooled_r = spool.tile([128, KC, T, 3], BF16, tag="pooled_r")
            nc.scalar.copy(pooled_r[:], pooled[:])
            nc.sync.dma_start(
                fbounce.ap().rearrange("(c p) f -> p c f", p=128), pooled_r[:])
            nc.gpsimd.collective_compute(
                "AllGather", ALU.bypass, replica_groups=[list(range(NC))],
                ins=[fbounce.ap().opt()], outs=[fgather.ap().opt()])
            feats = spool.tile([128, KC, T, B, 3], BF16, tag="feats")
            fg = fgather.ap().rearrange(
                "(b c p) (t st) -> c p t b st", b=B, c=KC, p=128, t=T, st=3)
            for c in range(KC):
                nc.sync.dma_start(feats[:, c], fg[c])

            # ---- fused (h0|c0 own chunk): [ROWS, 256] ----
            fu_ps = ppool.tile([128, 2 * CH], F32, tag="big")
            for c in range(KC):
                _mm(nc, fu_ps[:], feats[:, c, :, :, 1], wnf_sb[:, c],
                                 start=(c == 0), stop=False)
            for c in range(KC):
                _mm(nc, fu_ps[:], feats[:, c, :, :, 2], wvf_sb[:, c],
                                 start=False, stop=False)
            _mm(nc, fu_ps[:], ones_sb[:], bff_sb[:], start=False, stop=True)
            cst = spool.tile([128, CH], F32, tag="cst")      # cell state (full fp32)
            h_sb = spool.tile([128, CH], F32R, tag="h_sb")  # hidden (fp32r, feeds PE)
            nc.scalar.copy(h_sb[:], fu_ps[:, 0:CH])
            nc.vector.tensor_copy(cst[:], fu_ps[:, CH:])

            # ---- xpre (own gate cols, i|f|o|g): [ROWS, 512] ----
            xp_ps = ppool.tile([128, 4 * CH], F32, tag="big")
            for c in range(KC):
                _mm(nc, xp_ps[:], feats[:, c, :, :, 0], wih_sb[:, c],
                                 start=(c == 0), stop=False)
            _mm(nc, xp_ps[:], ones_sb[:], bihh_sb[:], start=False, stop=True)
            xpre = spool.tile([128, 4 * CH], F32R, tag="xpre")
            nc.vector.tensor_copy(xpre[:], xp_ps[:])

            # ---- h0 transpose -> hT_own ----
            hTown = spool.tile([128, 128], BF16, tag="hTown")
            t_ps = ppool.tile([128, 128], F32, tag="tps")
            _tr(nc, t_ps[:], h_sb[:], ident_sb[:])
            nc.scalar.copy(hTown[:], t_ps[:])

            # ---- phase C weights prefetch (overlaps LSTM) ----
            wc_sb = wpool.tile([128, KC, D], BF16, tag="wc")
            wa_sb = wpool.tile([128, KC, NA_PAD], F32R, tag="wa")
            bc_sb = wpool.tile([1, D], F32R, tag="bc")
            ba_sb = wpool.tile([1, NA_PAD], F32R, tag="ba")
            gam_sb = wpool.tile([128, D], F32, tag="gam")
            bet_sb = wpool.tile([128, D], F32, tag="bet")
            nc.sync.dma_start(wc_sb[:], wcT.ap().rearrange("k p n -> p k n"))
            nc.sync.dma_start(wa_sb[:], waT.ap().rearrange("k p n -> p k n"))
            nc.sync.dma_start(bc_sb[:], bcT.ap())
            nc.sync.dma_start(ba_sb[:], baT.ap())
            nc.sync.dma_start(gam_sb[:], gam.ap())
            nc.sync.dma_start(bet_sb[:], bet.ap())

            # ---- LSTM ----
            lasth = spool.tile([128, KC, 128], BF16, tag="lasth")  # lasth^T, all chunks
            for s in range(T + 1):
                Wd = _exw(s)
                Rs = ROWS - B * s     # active rows for compute step s
                # exchange hT chunks
                nc.sync.dma_start(ccin[s].ap(), hTown[:, 0:Wd])
                nc.gpsimd.collective_compute(
                    "AllGather", ALU.bypass, replica_groups=[list(range(NC))],
                    ins=[ccin[s].ap().opt()], outs=[ccout[s].ap().opt()])
                rr = ccout[s].ap().rearrange("(c p) w -> p c w", p=128)
                if s > 0:
                    # rows [Rs, Rs+B) retired after step s-1: capture their h^T
                    nc.sync.dma_start(lasth[:, :, Wd - B:Wd], rr[:, :, Wd - B:Wd])
                if s == T:
                    break
                hTall = kpool.tile([128, KC, Wd], BF16, tag="hTall")
                for c4 in range(4):
                    nc.sync.dma_start(hTall[:, 2 * c4:2 * c4 + 2, 0:Rs],
                                      rr[:, 2 * c4:2 * c4 + 2, 0:Rs])

                # gates: g = hT_all^T @ WhhT_own + xpre
                g_ps = ppool.tile([128, 4 * CH], F32, tag="gps")
                _mm(nc, g_ps[0:Rs], ident_sb[0:Rs, 0:Rs], xpre[0:Rs],
                    start=True, stop=False)
                for k in range(KC):
                    _mm(nc, g_ps[0:Rs], hTall[:, k, 0:Rs], whh_sb[:, k],
                                     start=False, stop=(k == KC - 1))
                act = kpool.tile([128, 4 * CH], F32, tag="act")
                nc.scalar.activation(act[0:Rs, 0:3 * CH], g_ps[0:Rs, 0:3 * CH], AF.Sigmoid)
                nc.scalar.activation(act[0:Rs, 3 * CH:], g_ps[0:Rs, 3 * CH:], AF.Tanh)
                # c = sf*c + si*tg ; h = so*tanh(c)
                t1 = kpool.tile([128, CH], F32, tag="t1")
                t2 = kpool.tile([128, CH], F32, tag="t2")
                nc.vector.tensor_mul(t1[0:Rs], act[0:Rs, 0:CH], act[0:Rs, 3 * CH:])
                nc.vector.tensor_mul(t2[0:Rs], act[0:Rs, CH:2 * CH], cst[0:Rs])
                nc.vector.tensor_add(cst[0:Rs], t1[0:Rs], t2[0:Rs])
                tc_t = kpool.tile([128, CH], F32, tag="tc")
                nc.scalar.activation(tc_t[0:Rs], cst[0:Rs], AF.Tanh)
                nc.vector.tensor_mul(h_sb[0:Rs], act[0:Rs, 2 * CH:3 * CH], tc_t[0:Rs])
                # transpose own h chunk
                t_ps2 = ppool.tile([128, 128], F32, tag="tps")
                _tr(nc, t_ps2[:, 0:Rs], h_sb[0:Rs], ident_sb[0:Rs, 0:Rs])
                nc.scalar.copy(hTown[:, 0:Rs], t_ps2[:, 0:Rs])

            # ---- phase C ----
            un_ps = ppool.tile([128, D], F32, tag="big")
            for n2 in range(2):
                nsl = slice(n2 * 512, (n2 + 1) * 512)
                for k in range(KC):
                    _mm(nc, un_ps[:, nsl], lasth[:, k], wc_sb[:, k, nsl],
                                     start=(k == 0), stop=False)
                _mm(nc, un_ps[:, nsl], ones_sb[:], bc_sb[:, nsl],
                                 start=False, stop=True)
            un_sb = kpool.tile([128, D], F32, tag="un_sb")
            sum_t = kpool.tile([128, 1], F32, tag="sum")
            nc.scalar.activation(un_sb[:], un_ps[:], AF.Copy, accum_out=sum_t[:])
            sq_sb = kpool.tile([128, D], F32, tag="sq")
            ssq_t = kpool.tile([128, 1], F32, tag="ssq")
            nc.scalar.activation(sq_sb[:], un_sb[:], AF.Square, accum_out=ssq_t[:])
            mean = kpool.tile([128, 1], F32, tag="mean")
            nc.vector.tensor_scalar_mul(mean[:], sum_t[:], 1.0 / D)
            em2 = kpool.tile([128, 1], F32, tag="em2")
            nc.vector.tensor_scalar_mul(em2[:], ssq_t[:], 1.0 / D)
            m2 = kpool.tile([128, 1], F32, tag="m2")
            nc.vector.tensor_mul(m2[:], mean[:], mean[:])
            var = kpool.tile([128, 1], F32, tag="var")
            nc.vector.tensor_sub(var[:], em2[:], m2[:])
            nc.vector.tensor_scalar_add(var[:], var[:], 1e-5)
            inv = kpool.tile([128, 1], F32, tag="inv")
            nc.vector.reciprocal(inv[:], var[:])
            istd = kpool.tile([128, 1], F32, tag="istd")
            nc.scalar.activation(istd[:], inv[:], AF.Sqrt)
            zn = kpool.tile([128, D], F32, tag="zn")
            nc.vector.tensor_scalar(zn[:], un_sb[:], mean[:], istd[:],
                                    op0=ALU.subtract, op1=ALU.mult)
            nc.vector.tensor_mul(zn[:], zn[:], gam_sb[:])
            nc.vector.tensor_add(zn[:], zn[:], bet_sb[:])
            relu = kpool.tile([128, D], F32R, tag="relu")
            nc.scalar.activation(relu[:], zn[:], AF.Relu)
            # transpose normed
            nT = kpool.tile([128, KC, 128], F32R, tag="nT")
            for k in range(KC):
                t_ps3 = ppool.tile([128, 128], F32, tag="tps")
                _tr(nc, t_ps3[:], relu[:, k * 128:(k + 1) * 128], ident_sb[:])
                nc.scalar.copy(nT[:, k], t_ps3[:])
            ao_ps = ppool.tile([128, NA_PAD], F32, tag="big")
            for k in range(KC):
                _mm(nc, ao_ps[:], nT[:, k], wa_sb[:, k],
                                 start=(k == 0), stop=False)
            _mm(nc, ao_ps[:], ones_sb[:], ba_sb[:], start=False, stop=True)
            out_sb = kpool.tile([128, NA_PAD], F32, tag="osb")
            nc.scalar.copy(out_sb[:], ao_ps[:])
            nc.sync.dma_start(out.ap(), out_sb[:])

    nc.compile()
    return nc


def _bf16(a):
    import ml_dtypes
    return np.ascontiguousarray(a).astype(ml_dtypes.bfloat16)


def _prep_inputs(inputs):
    """Host-side sharding + weight prep. Returns in_maps for 8 cores."""
    inp = {k: np.ascontiguousarray(np.asarray(v, np.float32)) for k, v in inputs.items()}
    x = inp["x"]
    scale = 1.0 / (H * W)

    Wf1 = inp["Wf"][:, :D]
    Wf2 = inp["Wf"][:, D:]
    Wnf = (Wf1 @ inp["Wn"]) * scale          # [2D, D]
    Wvf = (Wf2 @ inp["Wv"]) * scale
    bff = Wf1 @ inp["bn"] + Wf2 @ inp["bv"] + inp["bf"]
    Wih = inp["Wih"] * scale
    bihh = inp["bih"] + inp["bhh"]
    WhhT = inp["Whh"].T                      # [D, 4D]

    Wa_pad = np.zeros((NC * NA_PAD, D), np.float32)
    Wa_pad[:NUM_A] = inp["Wa"]
    ba_pad = np.zeros((NC * NA_PAD,), np.float32)
    ba_pad[:NUM_A] = inp["ba"]

    ident = np.eye(128, dtype=np.float32)
    ones = np.ones((1, 128), np.float32)
    wcT = np.ascontiguousarray(inp["Wc"].T.reshape(KC, 128, D))
    bcT = inp["bc"].reshape(1, D)
    gam_b = np.ascontiguousarray(np.broadcast_to(inp["g_a"], (128, D)))
    bet_b = np.ascontiguousarray(np.broadcast_to(inp["be_a"], (128, D)))

    in_maps = []
    for r in range(NC):
        hid = np.arange(r * CH, (r + 1) * CH)
        gsel = np.concatenate([0 * D + hid, 1 * D + hid, 3 * D + hid, 2 * D + hid])
        fsel = np.concatenate([hid, D + hid])
        m = {
            "xs": np.ascontiguousarray(x[r].reshape(T, D, HWST)),
            "ident": ident,
            "ones": ones,
            "wnfT": _bf16(np.ascontiguousarray(Wnf[fsel].T.reshape(KC, 128, 2 * CH))),
            "wvfT": _bf16(np.ascontiguousarray(Wvf[fsel].T.reshape(KC, 128, 2 * CH))),
            "bffT": np.ascontiguousarray(bff[fsel].reshape(1, 2 * CH)),
            "wihT": _bf16(np.ascontiguousarray(Wih[gsel].T.reshape(KC, 128, 4 * CH))),
            "bihhT": np.ascontiguousarray(bihh[gsel].reshape(1, 4 * CH)),
            "whhT": _bf16(np.ascontiguousarray(WhhT[:, gsel].reshape(KC, 128, 4 * CH))),
            "wcT": _bf16(wcT),
            "bcT": bcT,
            "waT": np.ascontiguousarray(
                Wa_pad[r * NA_PAD:(r + 1) * NA_PAD].T.reshape(KC, 128, NA_PAD)),
            "baT": np.ascontiguousarray(ba_pad[r * NA_PAD:(r + 1) * NA_PAD].reshape(1, NA_PAD)),
            "gam": gam_b,
            "bet": bet_b,
        }
        in_maps.append(m)
    return in_maps


def run_on_device(inputs, trace=False, **kwargs):
    if "nc" not in _CACHE:
        _CACHE["nc"] = build_program()
    nc = _CACHE["nc"]
    in_maps = _prep_inputs(inputs)
    res = run_bass_kernel_spmd(nc, in_maps, core_ids=list(range(NC)),
                               trace=trace, **kwargs)
    outs = [res.results[r]["out"] for r in range(NC)]
    full = np.concatenate(outs, axis=1)          # [ROWS, 1888]
    full = full[:, :NUM_A].reshape(T, B, NUM_A).transpose(1, 0, 2)
    return np.ascontiguousarray(full), res


def kernel(**inputs):
    out, _ = run_on_device(inputs)
    return out



# revision 7
# speedup vs baseline: 3.7381x; 3.7381x over previous
"""Trainium2 Bass kernel for nn_Classifier_52166672777735.

Strategy (8 NeuronCores, SPMD) — v2 "one-collective" design:
  - Pooling: D-sliced. Core r reads x[:, :, 128r:128(r+1)] (9.6MB) for ALL
    batches and pools over HxW -> pooled^T [128 D-part, 3 streams, 128 rows]
    (rows b-major).
  - Encoder: tensor-parallel partials. Core r holds the 128-row slice of
    [Wnf|Wvf|Wih] (2.1MB bf16) and computes partial (h0|c0|xpre) for all
    128 rows over its D-chunk -> [128 rows, 6144] bf16.
  - ONE ReduceScatter(add) sums partials and hands core r its own batch's
    16 rows: [16, 6144].  (The old design's 17 AllGathers at ~21us constant
    cost each were the dominant term; this design has exactly 1 collective.)
  - LSTM: row-sharded, fully local. Core r runs the full LSTM for batch r's
    16 rows with replicated Whh (8.4MB bf16). Gates computed TRANSPOSED
    ([128 gate-col part, rows free]) so the free dim = active rows (ragged
    prefix shrinks 16,15,...,1) and h lands directly in h^T layout for the
    next step's matmul — no per-step transpose, no exchange.
  - Classifier: replicated Wc/Wa (6MB bf16), LayerNorm per-row in
    [row-part, D-free] layout, final matmul transposed.
All re-layouts ([row, col] <-> [col, row]) are done with tiny identity
matmuls on the PE, never with strided DMA.
"""
import sys
import numpy as np

sys.path.insert(0, "/opt/trn_rl_repo")

from concourse import bass, bacc, tile, mybir  # noqa: E402
from concourse.bass_utils import run_bass_kernel_spmd  # noqa: E402

F32 = mybir.dt.float32
F32R = mybir.dt.float32r
BF16 = mybir.dt.bfloat16
AF = mybir.ActivationFunctionType
ALU = mybir.AluOpType

D = 1024
NUM_A = 1887
B, T, H, W = 8, 16, 7, 7
NC = 8
HWST = H * W * 3          # 147
ROWS = B * T              # 128, b-major (row = b*16 + t)
NA_PAD = 1920             # 15 * 128 >= 1887
ENC = 6144                # fused 2048 + gates 4096
NW = ENC // 512           # 12 encoder PSUM waves

_CACHE = {}


def _mm(nc, out, lhsT, rhs, **kw):
    if lhsT.dtype == F32:
        lhsT = lhsT.bitcast(F32R)
    if rhs.dtype == F32:
        rhs = rhs.bitcast(F32R)
    nc.tensor.matmul(out, lhsT, rhs, **kw)


def build_program(with_bias, with_gb):
    nc = bacc.Bacc("TRN2", target_bir_lowering=False, debug=False,
                   enable_asserts=True, num_devices=NC)

    # ---------------- I/O ----------------
    xs = nc.dram_tensor("xs", [B, T, 128, HWST], F32, kind="ExternalInput")
    wenc = nc.dram_tensor("wenc", [128, 8192], BF16, kind="ExternalInput")
    whhT = nc.dram_tensor("whhT", [8, 128, 4096], BF16, kind="ExternalInput")
    wcT = nc.dram_tensor("wcT", [8, 128, D], BF16, kind="ExternalInput")
    waT = nc.dram_tensor("waT", [8, 128, NA_PAD], BF16, kind="ExternalInput")
    ident = nc.dram_tensor("ident", [128, 128], F32R, kind="ExternalInput")
    id16 = nc.dram_tensor("id16", [16, 16], BF16, kind="ExternalInput")
    ones128 = nc.dram_tensor("ones128", [1, 128], F32R, kind="ExternalInput")
    ones16 = nc.dram_tensor("ones16", [1, 16], F32R, kind="ExternalInput")
    benc8 = nc.dram_tensor("benc8", [1, ENC], F32R, kind="ExternalInput")
    bcT = nc.dram_tensor("bcT", [1, D], F32R, kind="ExternalInput")
    baT = nc.dram_tensor("baT", [1, NA_PAD], F32R, kind="ExternalInput")
    gam = nc.dram_tensor("gam", [16, D], F32, kind="ExternalInput")
    bet = nc.dram_tensor("bet", [16, D], F32, kind="ExternalInput")
    out = nc.dram_tensor("out", [128, 15 * 16], F32, kind="ExternalOutput")

    pbounce = nc.dram_tensor("pbounce", [ROWS, ENC], BF16, kind="Internal")
    rsout = nc.dram_tensor("rsout", [T, ENC], BF16, kind="Internal")

    with tile.TileContext(nc) as tc:
        with (
            tc.tile_pool(name="w", bufs=1) as wpool,
            tc.tile_pool(name="xin", bufs=2) as xpool,
            tc.tile_pool(name="st", bufs=1) as spool,
            tc.tile_pool(name="wk", bufs=1) as kpool,
            tc.tile_pool(name="pn", bufs=3) as npool,
            tc.tile_pool(name="ps", bufs=2, space="PSUM") as ppool,
            tc.tile_pool(name="ps2", bufs=2, space="PSUM") as ppool2,
        ):
            # ---- tiny constants first (cheap DMA) ----
            ident_sb = wpool.tile([128, 128], F32R, tag="ident")
            id16_sb = wpool.tile([16, 16], BF16, tag="id16")
            ones128_sb = wpool.tile([1, 128], F32R, tag="on128")
            ones16_sb = wpool.tile([1, 16], F32R, tag="on16")
            nc.sync.dma_start(ident_sb[:], ident.ap())
            nc.sync.dma_start(id16_sb[:], id16.ap())
            nc.sync.dma_start(ones128_sb[:], ones128.ap())
            nc.sync.dma_start(ones16_sb[:], ones16.ap())
            if with_bias:
                benc_sb = wpool.tile([1, ENC], F32R, tag="benc")
                bc_sb = wpool.tile([1, D], F32R, tag="bc")
                ba_sb = wpool.tile([1, NA_PAD], F32R, tag="ba")
                nc.sync.dma_start(benc_sb[:], benc8.ap())
                nc.sync.dma_start(bc_sb[:], bcT.ap())
                nc.sync.dma_start(ba_sb[:], baT.ap())
            if with_gb:
                gam_sb = wpool.tile([16, D], F32, tag="gam")
                bet_sb = wpool.tile([16, D], F32, tag="bet")
                nc.sync.dma_start(gam_sb[:], gam.ap())
                nc.sync.dma_start(bet_sb[:], bet.ap())

            # ---- pooling: x (D-chunk, all batches) -> pooled^T ----
            pooled = spool.tile([128, 3, ROWS], F32, tag="pooled")
            for b in range(B):
                xt = xpool.tile([128, T, HWST], F32, tag="xt")
                nc.sync.dma_start(
                    xt[:], xs.ap()[b].rearrange("t p f -> p t f"))
                nc.vector.tensor_reduce(
                    pooled[:, :, 16 * b:16 * b + 16],
                    xt[:].rearrange("p t (hw st) -> p st t hw", st=3),
                    axis=mybir.AxisListType.X, op=ALU.add)

            # ---- encoder weights (after x on the DMA queue) ----
            wenc_sb = wpool.tile([128, 8192], BF16, tag="wenc")
            nc.sync.dma_start(wenc_sb[:], wenc.ap())

            pooled_b = spool.tile([128, 3, ROWS], BF16, tag="pooledb")
            nc.scalar.copy(pooled_b[:], pooled[:])

            # ---- encoder partials -> pbounce (bf16) ----
            for w in range(NW):
                eps = ppool.tile([128, 512], F32, tag="big")
                if w < 4:
                    sl = slice(512 * w, 512 * w + 512)
                    _mm(nc, eps[:], pooled_b[:, 1], wenc_sb[:, sl],
                        start=True, stop=False)
                    sl2 = slice(2048 + 512 * w, 2048 + 512 * w + 512)
                    _mm(nc, eps[:], pooled_b[:, 2], wenc_sb[:, sl2],
                        start=False, stop=not with_bias)
                else:
                    sl = slice(4096 + 512 * (w - 4), 4096 + 512 * (w - 4) + 512)
                    _mm(nc, eps[:], pooled_b[:, 0], wenc_sb[:, sl],
                        start=True, stop=not with_bias)
                if with_bias:
                    _mm(nc, eps[:], ones128_sb[:],
                        benc_sb[:, 512 * w:512 * w + 512],
                        start=False, stop=True)
                pc = npool.tile([128, 512], BF16, tag="penc")
                if w % 2 == 0:
                    nc.scalar.activation(pc[:], eps[:], AF.Copy)
                else:
                    nc.vector.tensor_copy(pc[:], eps[:])
                nc.sync.dma_start(pbounce.ap()[:, 512 * w:512 * w + 512], pc[:])

            # ---- the one collective ----
            nc.gpsimd.collective_compute(
                "ReduceScatter", ALU.add, replica_groups=[list(range(NC))],
                ins=[pbounce.ap().opt()], outs=[rsout.ap().opt()])

            # ---- big weights stream in while RS runs ----
            whh_sb = wpool.tile([128, 8, 4096], BF16, tag="whh")
            nc.sync.dma_start(whh_sb[:], whhT.ap().rearrange("k p n -> p k n"))
            wc_sb = wpool.tile([128, 8, D], BF16, tag="wc")
            nc.sync.dma_start(wc_sb[:], wcT.ap().rearrange("k p n -> p k n"))
            wa_sb = wpool.tile([128, 8, NA_PAD], BF16, tag="wa")
            nc.sync.dma_start(wa_sb[:], waT.ap().rearrange("k p n -> p k n"))

            # ---- RS result in (on ACT queue: skips the busy SP queue) ----
            rs_sb = spool.tile([16, ENC], BF16, tag="rs")
            nc.scalar.dma_start(rs_sb[:], rsout.ap())

            # ---- re-layout via identity matmuls ----
            # xpre^T: [128 gate-col, 32 grp x 16 rows]
            xp_ps = ppool.tile([128, 512], F32, tag="big")
            for g in range(32):
                _mm(nc, xp_ps[:, 16 * g:16 * g + 16],
                    rs_sb[:, 2048 + 128 * g:2048 + 128 * (g + 1)], id16_sb[:],
                    start=True, stop=True)
            xpreT = spool.tile([128, 512], F32R, tag="xpreT")
            nc.vector.tensor_copy(xpreT[:], xp_ps[:])
            # h0|c0 -> hT (bf16) and cst (f32), layout [128 hid, 8 chunk, 16 row]
            hc_ps = ppool.tile([128, 512], F32, tag="big")
            for g in range(16):
                _mm(nc, hc_ps[:, 16 * g:16 * g + 16],
                    rs_sb[:, 128 * g:128 * (g + 1)], id16_sb[:],
                    start=True, stop=True)
            hT = spool.tile([128, 8, 16], BF16, tag="hT")
            cst = spool.tile([128, 8, 16], F32, tag="cst")
            nc.scalar.copy(hT[:].rearrange("p k r -> p (k r)"), hc_ps[:, 0:128])
            nc.vector.tensor_copy(cst[:].rearrange("p k r -> p (k r)"),
                                  hc_ps[:, 128:256])

            # ---- LSTM: 16 steps, ragged active prefix, no exchanges ----
            for s in range(T):
                Rs = T - s
                g_ps = ppool2.tile([128, 512], F32, tag="gps")
                g3 = g_ps[:].rearrange("p (g r) -> p g r", r=16)
                _mm(nc, g_ps[:], ident_sb[:], xpreT[:], start=True, stop=False)
                for g in range(32):
                    for k in range(8):
                        _mm(nc, g_ps[:, 16 * g:16 * g + Rs],
                            whh_sb[:, k, 128 * g:128 * (g + 1)],
                            hT[:, k, 0:Rs], start=False, stop=(k == 7))
                sig = kpool.tile([128, 24, 16], F32, tag="sig")
                tg = kpool.tile([128, 8, 16], F32, tag="tg")
                nc.scalar.activation(sig[:, :, 0:Rs], g3[:, 0:24, 0:Rs],
                                     AF.Sigmoid)
                nc.scalar.activation(tg[:, :, 0:Rs], g3[:, 24:32, 0:Rs],
                                     AF.Tanh)
                t2 = kpool.tile([128, 8, 16], F32, tag="t2")
                t1 = kpool.tile([128, 8, 16], F32, tag="t1")
                nc.gpsimd.tensor_mul(t2[:, :, 0:Rs], sig[:, 8:16, 0:Rs],
                                     cst[:, :, 0:Rs])
                nc.vector.tensor_mul(t1[:, :, 0:Rs], sig[:, 0:8, 0:Rs],
                                     tg[:, :, 0:Rs])
                nc.vector.tensor_add(cst[:, :, 0:Rs], t1[:, :, 0:Rs],
                                     t2[:, :, 0:Rs])
                tc_t = kpool.tile([128, 8, 16], F32, tag="tc")
                nc.scalar.activation(tc_t[:, :, 0:Rs], cst[:, :, 0:Rs], AF.Tanh)
                nc.vector.tensor_mul(hT[:, :, 0:Rs], sig[:, 16:24, 0:Rs],
                                     tc_t[:, :, 0:Rs])

            # ---- classifier ----
            un_sb = kpool.tile([16, D], F32, tag="un")
            sum_t = kpool.tile([16, 1], F32, tag="sum")
            suma = kpool.tile([16, 1], F32, tag="suma")
            for h2 in range(2):
                upf = ppool.tile([128, 512], F32, tag="big")
                nsl = slice(512 * h2, 512 * h2 + 512)
                for k in range(8):
                    _mm(nc, upf[0:16, :], hT[:, k, :], wc_sb[:, k, nsl],
                        start=(k == 0), stop=(k == 7 and not with_bias))
                if with_bias:
                    _mm(nc, upf[0:16, :], ones16_sb[:], bc_sb[:, nsl],
                        start=False, stop=True)
                nc.scalar.activation(un_sb[:, nsl], upf[0:16, :], AF.Copy,
                                     accum_out=(sum_t[:] if h2 == 0
                                                else suma[:]))
            nc.vector.tensor_add(sum_t[:], sum_t[:], suma[:])
            sq_sb = kpool.tile([16, D], F32, tag="sq")
            ssq = kpool.tile([16, 1], F32, tag="ssq")
            nc.scalar.activation(sq_sb[:], un_sb[:], AF.Square,
                                 accum_out=ssq[:])
            mean = kpool.tile([16, 1], F32, tag="mean")
            nc.vector.tensor_scalar_mul(mean[:], sum_t[:], 1.0 / D)
            em2 = kpool.tile([16, 1], F32, tag="em2")
            nc.vector.tensor_scalar_mul(em2[:], ssq[:], 1.0 / D)
            m2 = kpool.tile([16, 1], F32, tag="m2")
            nc.vector.tensor_mul(m2[:], mean[:], mean[:])
            var = kpool.tile([16, 1], F32, tag="var")
            nc.vector.tensor_sub(var[:], em2[:], m2[:])
            nc.vector.tensor_scalar_add(var[:], var[:], 1e-5)
            inv = kpool.tile([16, 1], F32, tag="inv")
            nc.vector.reciprocal(inv[:], var[:])
            istd = kpool.tile([16, 1], F32, tag="istd")
            nc.scalar.activation(istd[:], inv[:], AF.Sqrt)
            zn = kpool.tile([16, D], F32, tag="zn")
            nc.vector.tensor_scalar(zn[:], un_sb[:], mean[:], istd[:],
                                    op0=ALU.subtract, op1=ALU.mult)
            if with_gb:
                nc.vector.tensor_mul(zn[:], zn[:], gam_sb[:])
                nc.vector.tensor_add(zn[:], zn[:], bet_sb[:])
            relu_b = kpool.tile([16, D], BF16, tag="relu")
            nc.scalar.activation(relu_b[:], zn[:], AF.Relu)
            # relu^T via identity matmuls
            rt_ps = ppool.tile([128, 512], F32, tag="big")
            for k in range(8):
                _mm(nc, rt_ps[:, 16 * k:16 * k + 16],
                    relu_b[:, 128 * k:128 * (k + 1)], id16_sb[:],
                    start=True, stop=True)
            reluT = kpool.tile([128, 8, 16], BF16, tag="reluT")
            nc.scalar.copy(reluT[:].rearrange("p k r -> p (k r)"), rt_ps[:, 0:128])
            # logits^T [128 class, 15 grp x 16 rows]
            ao_ps = ppool.tile([128, 512], F32, tag="big")
            for g in range(15):
                osl = slice(16 * g, 16 * g + 16)
                if with_bias:
                    _mm(nc, ao_ps[:, osl],
                        ba_sb[:, 128 * g:128 * (g + 1)], ones16_sb[:],
                        start=True, stop=False)
                for k in range(8):
                    _mm(nc, ao_ps[:, osl], wa_sb[:, k, 128 * g:128 * (g + 1)],
                        reluT[:, k, :],
                        start=(k == 0 and not with_bias), stop=(k == 7))
            out_sb = kpool.tile([128, 240], F32, tag="osb")
            nc.scalar.copy(out_sb[:], ao_ps[:, 0:240])
            nc.scalar.dma_start(out.ap(), out_sb[:])

    nc.compile()
    return nc


def _bf16(a):
    import ml_dtypes
    return np.ascontiguousarray(a).astype(ml_dtypes.bfloat16)


def _prep_inputs(inputs):
    inp = {k: np.ascontiguousarray(np.asarray(v, np.float32))
           for k, v in inputs.items()}
    x = inp["x"]
    scale = 1.0 / (H * W)

    Wf1 = inp["Wf"][:, :D]
    Wf2 = inp["Wf"][:, D:]
    Wnf = (Wf1 @ inp["Wn"]) * scale          # [2D, D]
    Wvf = (Wf2 @ inp["Wv"]) * scale
    bff = Wf1 @ inp["bn"] + Wf2 @ inp["bv"] + inp["bf"]
    WihS = inp["Wih"] * scale
    bihh = inp["bih"] + inp["bhh"]

    hid = np.arange(D)
    gsel = np.concatenate([0 * D + hid, 1 * D + hid, 3 * D + hid, 2 * D + hid])
    benc = np.concatenate([bff, bihh[gsel]]) / NC

    with_bias = bool(np.any(benc != 0.0) or np.any(inp["bc"] != 0.0)
                     or np.any(inp["ba"] != 0.0))
    with_gb = bool(np.any(inp["g_a"] != 1.0) or np.any(inp["be_a"] != 0.0))

    WnfT = Wnf.T                              # [D, 2048]
    WvfT = Wvf.T
    WihT = WihS[gsel].T                       # [D, 4096]
    whhT = _bf16(inp["Whh"].T[:, gsel].reshape(8, 128, 4096))
    wcT = _bf16(inp["Wc"].T.reshape(8, 128, D))
    Wa_pad = np.zeros((NA_PAD, D), np.float32)
    Wa_pad[:NUM_A] = inp["Wa"]
    waT = _bf16(Wa_pad.T.reshape(8, 128, NA_PAD))
    ba_pad = np.zeros((NA_PAD,), np.float32)
    ba_pad[:NUM_A] = inp["ba"]

    ident = np.eye(128, dtype=np.float32)
    id16 = np.eye(16, dtype=np.float32)
    gam16 = np.ascontiguousarray(np.broadcast_to(inp["g_a"], (16, D)))
    bet16 = np.ascontiguousarray(np.broadcast_to(inp["be_a"], (16, D)))

    in_maps = []
    for r in range(NC):
        dsl = slice(128 * r, 128 * (r + 1))
        wenc_r = np.concatenate([WnfT[dsl], WvfT[dsl], WihT[dsl]], axis=1)
        m = {
            "xs": np.ascontiguousarray(x[:, :, dsl].reshape(B, T, 128, HWST)),
            "wenc": _bf16(wenc_r),
            "whhT": whhT,
            "wcT": wcT,
            "waT": waT,
            "ident": ident,
            "id16": _bf16(id16),
            "ones128": np.ones((1, 128), np.float32),
            "ones16": np.ones((1, 16), np.float32),
            "benc8": benc.reshape(1, ENC).astype(np.float32),
            "bcT": inp["bc"].reshape(1, D),
            "baT": ba_pad.reshape(1, NA_PAD),
            "gam": gam16,
            "bet": bet16,
        }
        in_maps.append(m)
    return in_maps, with_bias, with_gb


def run_on_device(inputs, trace=False, **kwargs):
    in_maps, with_bias, with_gb = _prep_inputs(inputs)
    key = (with_bias, with_gb)
    if key not in _CACHE:
        _CACHE[key] = build_program(with_bias, with_gb)
    _CACHE["nc"] = _CACHE[key]
    nc = _CACHE[key]
    res = run_bass_kernel_spmd(nc, in_maps, core_ids=list(range(NC)),
                               trace=trace, **kwargs)
    full = np.empty((B, T, NUM_A), np.float32)
    for r in range(NC):
        o = res.results[r]["out"].reshape(128, 15, 16)
        full[r] = o.transpose(1, 0, 2).reshape(NA_PAD, 16)[:NUM_A].T
    return np.ascontiguousarray(full), res


def kernel(**inputs):
    out, _ = run_on_device(inputs)
    return out
